# revision 13
# baseline (speedup 1.0000x reference)
"""Cross-level attention (3 KV levels: causal T=2048, full T1=512, full T2=128)
for B=2, H=16, T=2048, DH=64 on 8 Trainium2 NeuronCores.

Sharding: the 32 (b, h) pairs are split 4-per-core (batch + head parallel);
each core computes full attention for its 4 heads, level_w replicated.

Per-core dataflow (all operands resident in SBUF after one initial load):
  - Heads are processed as 2 "duos" (2 pairs packed on partitions 0-63 /
    64-127) so the two K=64-contraction QK^T matmuls occupy disjoint PE row
    groups and run concurrently.
  - S^T tiles [s=128, t=512] are computed per 128-wide K-block j via
    matmul(lhsT=K^T, rhs=Q^T); exp (with the 1/sqrt(dh) scale folded in) runs
    on the Scalar engine straight out of PSUM into bf16 SBUF tiles P^T.
  - Causal masking for level 0 is block-level: fully-masked blocks are
    skipped, diagonal blocks get a 0/1 upper-triangular multiply post-exp.
  - PV: matmul(lhsT=P^T[:, c-slice], rhs=V'[s-block]) accumulating in PSUM
    over s-blocks, where V' carries a ones-column so the softmax denominator
    accumulates in output column 64 for free.
  - Combine: out = sum_l w_l / rowsum_l * PV_l  (per-partition scalars on the
    Vector engine), DMA to DRAM.
"""
import numpy as np
import ml_dtypes

B, H, T, DH = 2, 16, 2048, 64
C = H * DH
T1, T2 = 512, 128
NCORES = 8
PAIRS = 4          # (b, h) pairs per core
DUOS = 2           # pairs are packed two-per-SBUF-tile
QB = T // 512      # 512-wide query blocks
NJ = (T // 128, T1 // 128, T2 // 128)

TRACE = False          # set by test.py for profiling runs
LAST_RESULT = None     # BassKernelResults from the most recent run

_NC_CACHE = {}


def _build_nc(w):
    import concourse.bass as bass
    from concourse import bacc
    import concourse.tile as tile
    import concourse.mybir as mybir
    from contextlib import ExitStack

    BF16 = mybir.dt.bfloat16
    F32 = mybir.dt.float32
    Exp = mybir.ActivationFunctionType.Exp

    nc = bacc.Bacc("TRN2", target_bir_lowering=False)
    qt = nc.dram_tensor("qt", [DUOS, 128, T], BF16, kind="ExternalInput")
    k0t = nc.dram_tensor("k0t", [DUOS, 128, T], BF16, kind="ExternalInput")
    k1t = nc.dram_tensor("k1t", [DUOS, 128, T1], BF16, kind="ExternalInput")
    k2t = nc.dram_tensor("k2t", [DUOS, 128, T2], BF16, kind="ExternalInput")
    v0 = nc.dram_tensor("v0", [PAIRS, 128, NJ[0], 65], BF16, kind="ExternalInput")
    v1 = nc.dram_tensor("v1", [PAIRS, 128, NJ[1], 65], BF16, kind="ExternalInput")
    v2 = nc.dram_tensor("v2", [PAIRS, 128, NJ[2], 65], BF16, kind="ExternalInput")
    tri = nc.dram_tensor("tri", [128, 128], BF16, kind="ExternalInput")
    out = nc.dram_tensor("out", [PAIRS, T // 128, 128, DH], F32, kind="ExternalOutput")

    with tile.TileContext(nc) as tc, ExitStack() as ctx:
        const = ctx.enter_context(tc.tile_pool(name="const", bufs=1))
        qkp = ctx.enter_context(tc.tile_pool(name="qkp", bufs=2, space="PSUM"))
        pvp = ctx.enter_context(tc.tile_pool(name="pvp", bufs=4, space="PSUM"))
        pts = ctx.enter_context(tc.tile_pool(name="pts", bufs=44))
        outp = ctx.enter_context(tc.tile_pool(name="outp", bufs=4))
        small = ctx.enter_context(tc.tile_pool(name="small", bufs=8))

        def load(dram_ap, shape, tag):
            t = const.tile(shape, BF16, tag=tag, name=tag)
            nc.sync.dma_start(out=t, in_=dram_ap)
            return t

        # Big Q/K tensors are DMA'd in 512-column chunks so the first
        # query-block's matmuls can start before the full tensors land.
        def load_chunked(dram_ap, shape, tag, chunk=512):
            t = const.tile(shape, BF16, tag=tag, name=tag)
            for c0 in range(0, shape[1], chunk):
                nc.sync.dma_start(out=t[:, c0:c0 + chunk],
                                  in_=dram_ap[:, c0:c0 + chunk])
            return t

        sb_tri = load(tri[:], [128, 128], "tri")
        sb_qt = [load_chunked(qt[d], [128, T], f"qt{d}") for d in range(DUOS)]
        sb_kt = [
            [load_chunked(k0t[d], [128, T], f"k0t{d}") for d in range(DUOS)],
            [load(k1t[d], [128, T1], f"k1t{d}") for d in range(DUOS)],
            [load(k2t[d], [128, T2], f"k2t{d}") for d in range(DUOS)],
        ]
        sb_v = [
            [load(v0[p], [128, NJ[0], 65], f"v0{p}") for p in range(PAIRS)],
            [load(v1[p], [128, NJ[1], 65], f"v1{p}") for p in range(PAIRS)],
            [load(v2[p], [128, NJ[2], 65], f"v2{p}") for p in range(PAIRS)],
        ]

        def unit_jblocks(qb):
            return ([(0, j) for j in range(4 * qb + 4)]
                    + [(1, j) for j in range(NJ[1])]
                    + [(2, j) for j in range(NJ[2])])

        def phase1(d, qb):
                jblocks = unit_jblocks(qb)
                # ---- Phase 1: S^T = K^T.T @ Q^T per j-block, exp -> P^T
                pt_tiles = {}
                for (l, j) in jblocks:
                    # Diagonal L0 blocks: columns left of the diagonal 128-col
                    # sub-block are fully causal-masked and never read by PV,
                    # so compute/exp only the [f0, 512) column range.
                    f0 = 128 * (j - 4 * qb) if (l == 0 and j >= 4 * qb) else 0
                    sp = qkp.tile([128, 2, 512], F32, tag="qk", name="qk")
                    for half in range(2):
                        nc.tensor.matmul(
                            out=sp[:, half, f0:],
                            lhsT=sb_kt[l][d][64 * half:64 * half + 64,
                                             128 * j:128 * j + 128],
                            rhs=sb_qt[d][64 * half:64 * half + 64,
                                         512 * qb + f0:512 * qb + 512],
                            start=True, stop=True,
                        )
                    pt = pts.tile([128, 2, 512], BF16, tag="pt", name="pt")
                    nc.scalar.activation(out=pt[:, :, f0:], in_=sp[:, :, f0:],
                                         func=Exp, scale=DH ** -0.5)
                    if l == 0 and j >= 4 * qb:
                        for half in range(2):
                            s = pt[:, half, f0:f0 + 128]
                            nc.vector.tensor_mul(out=s, in0=s, in1=sb_tri)
                    pt_tiles[(l, j)] = pt
                return pt_tiles

        def phase2(d, qb, pt_tiles):
                jblocks = unit_jblocks(qb)
                # ---- Phase 2: PV accumulation + combine, per pair half
                for half in range(2):
                    p = 2 * d + half
                    pvt = [pvp.tile([128, 4, 65], F32, tag="pv", name=f"pv{_l}") for _l in range(3)]
                    for (l, j) in jblocks:
                        pt = pt_tiles[(l, j)]
                        # One accumulation group per PSUM bank (= per level):
                        # start zeroes the whole 2KB zero region, so only the
                        # very first matmul into the bank may set start=True.
                        lvl_last_j = (4 * qb + 3) if l == 0 else NJ[l] - 1
                        for c in range(4):
                            if l == 0 and j > 4 * qb + c:
                                continue
                            nc.tensor.matmul(
                                out=pvt[l][:, c, :],
                                lhsT=pt[:, half, 128 * c:128 * c + 128],
                                rhs=sb_v[l][p][:, j, :],
                                start=(j == 0 and c == 0),
                                stop=(j == lvl_last_j and c == 3),
                            )
                    osb = outp.tile([128, 4, DH], F32, tag="osb", name="osb")
                    for l in range(3):
                        rc = small.tile([128, 4, 1], F32, tag="rc", name="rc")
                        nc.vector.reciprocal(out=rc[:, :, 0], in_=pvt[l][:, :, 64])
                        dst = osb if l == 0 else outp.tile([128, 4, DH], F32, tag="tmp", name="tmp")
                        nc.vector.tensor_mul(
                            out=dst, in0=pvt[l][:, :, 0:64],
                            in1=rc.broadcast_to([128, 4, DH]))
                        if l > 0:
                            nc.gpsimd.tensor_add(out=osb, in0=osb, in1=dst)
                    for c in range(4):
                        nc.sync.dma_start(out=out[p, 4 * qb + c], in_=osb[:, c, :])

        # Software pipeline: run unit u's QK+exp before unit u-1's PV, so the
        # Scalar engine always has a full unit of exp work queued while the
        # PE drains the previous unit's PV backlog. Unit order puts a small
        # unit first (fast start after partial DMA) and a small-ish one last
        # (short PV tail after the final exp).
        units = [(d, qb) for d in range(DUOS) for qb in (0, 3, 2, 1)]
        prev = None
        for u in units:
            pt_tiles = phase1(*u)
            if prev is not None:
                phase2(prev[0][0], prev[0][1], prev[1])
            prev = (u, pt_tiles)
        phase2(prev[0][0], prev[0][1], prev[1])
    nc.compile()
    return nc


def _prepare(inputs):
    bf = ml_dtypes.bfloat16
    Q = np.asarray(inputs["Q"], np.float32)
    Ks = [np.asarray(inputs[k], np.float32) for k in ("K0", "K1", "K2")]
    Vs = [np.asarray(inputs[k], np.float32) for k in ("V0", "V1", "V2")]
    level_w = np.asarray(inputs["level_w"], np.float64)
    e = np.exp(level_w - level_w.max())
    w = (e / e.sum()).astype(np.float64)

    # Host-side layout for sharding: per-head transposed Q/K ([64, Tm]) and
    # s-tiled V with a ones column ([128, nj, 65]).
    QT = np.ascontiguousarray(Q.transpose(0, 1, 3, 2)).astype(bf)  # [B,H,64,T]
    KTs = []
    for Kl in Ks:
        Tm = Kl.shape[1]
        Kh = Kl.reshape(B, Tm, H, DH).transpose(0, 2, 3, 1)  # [B,H,64,Tm]
        KTs.append(np.ascontiguousarray(Kh).astype(bf))
    Vps = []
    for lvl, Vl in enumerate(Vs):
        Tm = Vl.shape[1]
        Vl = Vl * np.float32(w[lvl])   # fold level weight into V (exact in fp32)
        Vh = Vl.reshape(B, Tm, H, DH).transpose(0, 2, 1, 3)  # [B,H,Tm,64]
        vp = np.ones((B, H, Tm // 128, 128, 65), np.float32)
        vp[..., :64] = Vh.reshape(B, H, Tm // 128, 128, DH)
        # -> [B, H, 128(p), nj, 65]
        Vps.append(np.ascontiguousarray(vp.transpose(0, 1, 3, 2, 4)).astype(bf))
    tri = (np.arange(128)[:, None] <= np.arange(128)[None, :]).astype(bf)

    in_maps = []
    for core in range(NCORES):
        m = {
            "qt": np.empty((DUOS, 128, T), bf),
            "k0t": np.empty((DUOS, 128, T), bf),
            "k1t": np.empty((DUOS, 128, T1), bf),
            "k2t": np.empty((DUOS, 128, T2), bf),
            "v0": np.empty((PAIRS, 128, NJ[0], 65), bf),
            "v1": np.empty((PAIRS, 128, NJ[1], 65), bf),
            "v2": np.empty((PAIRS, 128, NJ[2], 65), bf),
            "tri": tri,
        }
        for p in range(PAIRS):
            g = PAIRS * core + p
            b, h = divmod(g, H)
            d, half = divmod(p, 2)
            sl = slice(64 * half, 64 * half + 64)
            m["qt"][d, sl] = QT[b, h]
            m["k0t"][d, sl] = KTs[0][b, h]
            m["k1t"][d, sl] = KTs[1][b, h]
            m["k2t"][d, sl] = KTs[2][b, h]
            m["v0"][p] = Vps[0][b, h]
            m["v1"][p] = Vps[1][b, h]
            m["v2"][p] = Vps[2][b, h]
        in_maps.append(m)

    return in_maps, w


def kernel(**inputs):
    global LAST_RESULT
    from concourse.bass_utils import run_bass_kernel_spmd

    in_maps, w = _prepare(inputs)
    key = tuple(np.asarray(w, np.float64).tolist())
    if key not in _NC_CACHE:
        _NC_CACHE[key] = _build_nc(w)
    nc = _NC_CACHE[key]

    try:
        res = run_bass_kernel_spmd(nc, in_maps, core_ids=list(range(NCORES)),
                                   trace=TRACE)
    except (ImportError, ModuleNotFoundError):
        # axon build without the NTFF profiling hook — run without trace
        res = run_bass_kernel_spmd(nc, in_maps, core_ids=list(range(NCORES)),
                                   trace=False)
    LAST_RESULT = res

    outs = np.stack([np.asarray(r["out"]) for r in res.results])  # [8,4,16,128,64]
    O = outs.reshape(B, H, T, DH)           # pair-major == (b, h)-major
    return np.ascontiguousarray(O.transpose(0, 2, 1, 3)).reshape(B, T, C).astype(np.float32)


# revision 17
# speedup vs baseline: 1.0995x; 1.0995x over previous
"""Cross-level attention (3 KV levels: causal T=2048, full T1=512, full T2=128)
for B=2, H=16, T=2048, DH=64 on 8 Trainium2 NeuronCores.

Sharding: the 32 (b, h) pairs are split 4-per-core (batch + head parallel);
each core computes full attention for its 4 heads, level_w replicated.

Per-core dataflow (all operands resident in SBUF after one initial load):
  - Heads are processed as 2 "duos" (2 pairs packed on partitions 0-63 /
    64-127) so the two K=64-contraction QK^T matmuls occupy disjoint PE row
    groups and run concurrently.
  - S^T tiles [s=128, t=512] are computed per 128-wide K-block j via
    matmul(lhsT=K^T, rhs=Q^T); exp (with the 1/sqrt(dh) scale folded in) runs
    on the Scalar engine straight out of PSUM into bf16 SBUF tiles P^T.
  - Causal masking for level 0 is block-level: fully-masked blocks are
    skipped (in both compute and exp), diagonal blocks get a 0/1
    upper-triangular multiply post-exp.
  - PV: matmul(lhsT=P^T[:, c-slice], rhs=V'[s-block]) accumulating in PSUM
    over s-blocks, where V' carries a ones-column so the softmax denominator
    accumulates in output column 64 for free. Level weights w_l are folded
    into V on the host, so combine is out = sum_l PV_l / rowsum_l
    (reciprocal + per-column broadcast multiply on DVE, adds on Pool).
  - The per-(duo, query-block) units are software-pipelined one deep
    (QK+exp of unit u before PV of unit u-1) so the Scalar engine — the
    bottleneck at ~1 exp/lane/cycle — never starves while the PE drains the
    PV backlog. Unit order and 512-column-chunked input DMAs minimize the
    startup and drain tails.

TimelineSim cost model: ~130 us/core (Scalar/exp-bound; PE ~72 us,
DVE ~33 us, Pool ~32 us busy).
"""
import numpy as np
import ml_dtypes

B, H, T, DH = 2, 16, 2048, 64
C = H * DH
T1, T2 = 512, 128
NCORES = 8
PAIRS = 4          # (b, h) pairs per core
DUOS = 2           # pairs are packed two-per-SBUF-tile
QB = T // 512      # 512-wide query blocks
NJ = (T // 128, T1 // 128, T2 // 128)

TRACE = False          # set by test.py for profiling runs
LAST_RESULT = None     # BassKernelResults from the most recent run

_NC_CACHE = {}


def _build_nc(w):
    import concourse.bass as bass
    from concourse import bacc
    import concourse.tile as tile
    import concourse.mybir as mybir
    from contextlib import ExitStack

    BF16 = mybir.dt.bfloat16
    F32 = mybir.dt.float32
    Exp = mybir.ActivationFunctionType.Exp

    nc = bacc.Bacc("TRN2", target_bir_lowering=False)
    qt = nc.dram_tensor("qt", [DUOS, 128, T], BF16, kind="ExternalInput")
    k0t = nc.dram_tensor("k0t", [DUOS, 128, T], BF16, kind="ExternalInput")
    k1t = nc.dram_tensor("k1t", [DUOS, 128, T1], BF16, kind="ExternalInput")
    k2t = nc.dram_tensor("k2t", [DUOS, 128, T2], BF16, kind="ExternalInput")
    v0 = nc.dram_tensor("v0", [PAIRS, 128, NJ[0], 65], BF16, kind="ExternalInput")
    v1 = nc.dram_tensor("v1", [PAIRS, 128, NJ[1], 65], BF16, kind="ExternalInput")
    v2 = nc.dram_tensor("v2", [PAIRS, 128, NJ[2], 65], BF16, kind="ExternalInput")
    tri = nc.dram_tensor("tri", [128, 128], BF16, kind="ExternalInput")
    out = nc.dram_tensor("out", [PAIRS, 128, T // 128, DH], F32, kind="ExternalOutput")

    with tile.TileContext(nc) as tc, ExitStack() as ctx:
        const = ctx.enter_context(tc.tile_pool(name="const", bufs=1))
        qkp = ctx.enter_context(tc.tile_pool(name="qkp", bufs=2, space="PSUM"))
        pvp = ctx.enter_context(tc.tile_pool(name="pvp", bufs=4, space="PSUM"))
        pts = ctx.enter_context(tc.tile_pool(name="pts", bufs=44))
        outp = ctx.enter_context(tc.tile_pool(name="outp", bufs=4))
        small = ctx.enter_context(tc.tile_pool(name="small", bufs=8))

        def load(dram_ap, shape, tag):
            t = const.tile(shape, BF16, tag=tag, name=tag)
            nc.sync.dma_start(out=t, in_=dram_ap)
            return t

        # Big Q/K tensors are DMA'd in 512-column chunks so the first
        # query-block's matmuls can start before the full tensors land.
        def load_chunked(dram_ap, shape, tag, chunk=512):
            t = const.tile(shape, BF16, tag=tag, name=tag)
            for c0 in range(0, shape[1], chunk):
                nc.sync.dma_start(out=t[:, c0:c0 + chunk],
                                  in_=dram_ap[:, c0:c0 + chunk])
            return t

        # Emission order = need order: the first unit is (d=0, qb=0), which
        # needs only qt0/k0t0 column chunk 0 plus k1t/k2t of duo 0. V tiles go
        # on the gpsimd SWDGE queue so they stream in parallel with the
        # HWDGE-queued Q/K chunks.
        sb_tri = load(tri[:], [128, 128], "tri")
        def alloc(shape, tag):
            return const.tile(shape, BF16, tag=tag, name=tag)
        sb_qt = [alloc([128, T], f"qt{d}") for d in range(DUOS)]
        sb_kt = [[alloc([128, T], f"k0t{d}") for d in range(DUOS)],
                 [alloc([128, T1], f"k1t{d}") for d in range(DUOS)],
                 [alloc([128, T2], f"k2t{d}") for d in range(DUOS)]]
        sb_v = [[alloc([128, NJ[0], 65], f"v0{p}") for p in range(PAIRS)],
                [alloc([128, NJ[1], 65], f"v1{p}") for p in range(PAIRS)],
                [alloc([128, NJ[2], 65], f"v2{p}") for p in range(PAIRS)]]

        def dma_cols(t, dram_ap, c0, c1, engine=None):
            (engine or nc.sync).dma_start(out=t[:, c0:c1], in_=dram_ap[:, c0:c1])

        for d in range(DUOS):
            # unit (d, 0): first 512 cols of qt/k0t + all of k1t/k2t
            dma_cols(sb_qt[d], qt[d], 0, 512)
            dma_cols(sb_kt[0][d], k0t[d], 0, 512)
            nc.sync.dma_start(out=sb_kt[1][d], in_=k1t[d])
            nc.sync.dma_start(out=sb_kt[2][d], in_=k2t[d])
            for p in (2 * d, 2 * d + 1):
                nc.gpsimd.dma_start(out=sb_v[0][p], in_=v0[p])
                nc.gpsimd.dma_start(out=sb_v[1][p], in_=v1[p])
                nc.gpsimd.dma_start(out=sb_v[2][p], in_=v2[p])
            # remaining qt/k0t chunks (needed from unit (d, 3) onwards)
            for c0 in range(512, T, 512):
                dma_cols(sb_kt[0][d], k0t[d], c0, c0 + 512)
            for c0 in range(512, T, 512):
                dma_cols(sb_qt[d], qt[d], c0, c0 + 512)

        def unit_jblocks(qb):
            return ([(0, j) for j in range(4 * qb + 4)]
                    + [(1, j) for j in range(NJ[1])]
                    + [(2, j) for j in range(NJ[2])])

        def phase1(d, qb):
                jblocks = unit_jblocks(qb)
                # ---- Phase 1: S^T = K^T.T @ Q^T per j-block, exp -> P^T
                pt_tiles = {}
                for (l, j) in jblocks:
                    # Diagonal L0 blocks: columns left of the diagonal 128-col
                    # sub-block are fully causal-masked and never read by PV,
                    # so compute/exp only the [f0, 512) column range.
                    f0 = 128 * (j - 4 * qb) if (l == 0 and j >= 4 * qb) else 0
                    sp = qkp.tile([128, 2, 512], F32, tag="qk", name="qk")
                    for half in range(2):
                        nc.tensor.matmul(
                            out=sp[:, half, f0:],
                            lhsT=sb_kt[l][d][64 * half:64 * half + 64,
                                             128 * j:128 * j + 128],
                            rhs=sb_qt[d][64 * half:64 * half + 64,
                                         512 * qb + f0:512 * qb + 512],
                            start=True, stop=True,
                        )
                    pt = pts.tile([128, 2, 512], BF16, tag="pt", name="pt")
                    nc.scalar.activation(out=pt[:, :, f0:], in_=sp[:, :, f0:],
                                         func=Exp, scale=DH ** -0.5)
                    if l == 0 and j >= 4 * qb:
                        for half in range(2):
                            s = pt[:, half, f0:f0 + 128]
                            nc.vector.tensor_mul(out=s, in0=s, in1=sb_tri)
                    pt_tiles[(l, j)] = pt
                return pt_tiles

        def phase2(d, qb, pt_tiles):
                jblocks = unit_jblocks(qb)
                # ---- Phase 2: PV accumulation + combine, per pair half
                for half in range(2):
                    p = 2 * d + half
                    pvt = [pvp.tile([128, 4, 65], F32, tag="pv", name=f"pv{_l}") for _l in range(3)]
                    for (l, j) in jblocks:
                        pt = pt_tiles[(l, j)]
                        # One accumulation group per PSUM bank (= per level):
                        # start zeroes the whole 2KB zero region, so only the
                        # very first matmul into the bank may set start=True.
                        lvl_last_j = (4 * qb + 3) if l == 0 else NJ[l] - 1
                        for c in range(4):
                            if l == 0 and j > 4 * qb + c:
                                continue
                            nc.tensor.matmul(
                                out=pvt[l][:, c, :],
                                lhsT=pt[:, half, 128 * c:128 * c + 128],
                                rhs=sb_v[l][p][:, j, :],
                                start=(j == 0 and c == 0),
                                stop=(j == lvl_last_j and c == 3),
                            )
                    osb = outp.tile([128, 4, DH], F32, tag="osb", name="osb")
                    for l in range(3):
                        rc = small.tile([128, 4, 1], F32, tag="rc", name="rc")
                        nc.vector.reciprocal(out=rc[:, :, 0], in_=pvt[l][:, :, 64])
                        dst = osb if l == 0 else outp.tile([128, 4, DH], F32, tag="tmp", name="tmp")
                        nc.vector.tensor_mul(
                            out=dst, in0=pvt[l][:, :, 0:64],
                            in1=rc.broadcast_to([128, 4, DH]))
                        if l > 0:
                            nc.gpsimd.tensor_add(out=osb, in0=osb, in1=dst)
                    nc.sync.dma_start(out=out[p][:, 4 * qb:4 * qb + 4, :],
                                      in_=osb)

        # Software pipeline: run unit u's QK+exp before unit u-1's PV, so the
        # Scalar engine always has a full unit of exp work queued while the
        # PE drains the previous unit's PV backlog. Unit order puts a small
        # unit first (fast start after partial DMA) and a small-ish one last
        # (short PV tail after the final exp).
        units = [(0, 0), (0, 3), (0, 2), (0, 1), (1, 3), (1, 2), (1, 1), (1, 0)]
        tiles = {}
        pending = []
        for i, u in enumerate(units):
            tiles[u] = phase1(*u)
            if i < len(units) - 2:
                if pending:
                    v = pending.pop(0)
                    phase2(v[0], v[1], tiles.pop(v))
                pending.append(u)
            else:
                # run the last two units' QK+exp back-to-back so the Scalar
                # engine has a full runway while the PE drains the PV backlog
                pending.append(u)
        for v in pending:
            phase2(v[0], v[1], tiles.pop(v))
    nc.compile()
    return nc


def _prepare(inputs):
    bf = ml_dtypes.bfloat16
    Q = np.asarray(inputs["Q"], np.float32)
    Ks = [np.asarray(inputs[k], np.float32) for k in ("K0", "K1", "K2")]
    Vs = [np.asarray(inputs[k], np.float32) for k in ("V0", "V1", "V2")]
    level_w = np.asarray(inputs["level_w"], np.float64)
    e = np.exp(level_w - level_w.max())
    w = (e / e.sum()).astype(np.float64)

    # Host-side layout for sharding: per-head transposed Q/K ([64, Tm]) and
    # s-tiled V with a ones column ([128, nj, 65]).
    QT = np.ascontiguousarray(Q.transpose(0, 1, 3, 2)).astype(bf)  # [B,H,64,T]
    KTs = []
    for Kl in Ks:
        Tm = Kl.shape[1]
        Kh = Kl.reshape(B, Tm, H, DH).transpose(0, 2, 3, 1)  # [B,H,64,Tm]
        KTs.append(np.ascontiguousarray(Kh).astype(bf))
    Vps = []
    for lvl, Vl in enumerate(Vs):
        Tm = Vl.shape[1]
        Vl = Vl * np.float32(w[lvl])   # fold level weight into V (exact in fp32)
        Vh = Vl.reshape(B, Tm, H, DH).transpose(0, 2, 1, 3)  # [B,H,Tm,64]
        vp = np.ones((B, H, Tm // 128, 128, 65), np.float32)
        vp[..., :64] = Vh.reshape(B, H, Tm // 128, 128, DH)
        # -> [B, H, 128(p), nj, 65]
        Vps.append(np.ascontiguousarray(vp.transpose(0, 1, 3, 2, 4)).astype(bf))
    tri = (np.arange(128)[:, None] <= np.arange(128)[None, :]).astype(bf)

    in_maps = []
    for core in range(NCORES):
        m = {
            "qt": np.empty((DUOS, 128, T), bf),
            "k0t": np.empty((DUOS, 128, T), bf),
            "k1t": np.empty((DUOS, 128, T1), bf),
            "k2t": np.empty((DUOS, 128, T2), bf),
            "v0": np.empty((PAIRS, 128, NJ[0], 65), bf),
            "v1": np.empty((PAIRS, 128, NJ[1], 65), bf),
            "v2": np.empty((PAIRS, 128, NJ[2], 65), bf),
            "tri": tri,
        }
        for p in range(PAIRS):
            g = PAIRS * core + p
            b, h = divmod(g, H)
            d, half = divmod(p, 2)
            sl = slice(64 * half, 64 * half + 64)
            m["qt"][d, sl] = QT[b, h]
            m["k0t"][d, sl] = KTs[0][b, h]
            m["k1t"][d, sl] = KTs[1][b, h]
            m["k2t"][d, sl] = KTs[2][b, h]
            m["v0"][p] = Vps[0][b, h]
            m["v1"][p] = Vps[1][b, h]
            m["v2"][p] = Vps[2][b, h]
        in_maps.append(m)

    return in_maps, w


def kernel(**inputs):
    global LAST_RESULT
    from concourse.bass_utils import run_bass_kernel_spmd

    in_maps, w = _prepare(inputs)
    key = tuple(np.asarray(w, np.float64).tolist())
    if key not in _NC_CACHE:
        _NC_CACHE[key] = _build_nc(w)
    nc = _NC_CACHE[key]

    try:
        res = run_bass_kernel_spmd(nc, in_maps, core_ids=list(range(NCORES)),
                                   trace=TRACE)
    except (ImportError, ModuleNotFoundError):
        # axon build without the NTFF profiling hook — run without trace
        res = run_bass_kernel_spmd(nc, in_maps, core_ids=list(range(NCORES)),
                                   trace=False)
    LAST_RESULT = res

    outs = np.stack([np.asarray(r["out"]) for r in res.results])  # [8,4,128,16,64]
    O = outs.transpose(0, 1, 3, 2, 4).reshape(B, H, T, DH)  # t = 128*n + pp
    return np.ascontiguousarray(O.transpose(0, 2, 1, 3)).reshape(B, T, C).astype(np.float32)


# revision 20
# speedup vs baseline: 1.1039x; 1.0040x over previous
"""Cross-level attention (3 KV levels: causal T=2048, full T1=512, full T2=128)
for B=2, H=16, T=2048, DH=64 on 8 Trainium2 NeuronCores.

Sharding: the 32 (b, h) pairs are split 4-per-core (batch + head parallel);
each core computes full attention for its 4 heads, level_w replicated.

Per-core dataflow (all operands resident in SBUF after one initial load):
  - Heads are processed as 2 "duos" (2 pairs packed on partitions 0-63 /
    64-127) so the two K=64-contraction QK^T matmuls occupy disjoint PE row
    groups and run concurrently.
  - S^T tiles [s=128, t=512] are computed per 128-wide K-block j via
    matmul(lhsT=K^T, rhs=Q^T); exp (with the 1/sqrt(dh) scale folded in) runs
    on the Scalar engine straight out of PSUM into bf16 SBUF tiles P^T.
  - Causal masking for level 0 is block-level: fully-masked blocks are
    skipped (in both compute and exp), diagonal blocks get a 0/1
    upper-triangular multiply post-exp.
  - PV: matmul(lhsT=P^T[:, c-slice], rhs=V'[s-block]) accumulating in PSUM
    over s-blocks, where V' carries a ones-column so the softmax denominator
    accumulates in output column 64 for free. Level weights w_l are folded
    into V on the host, so combine is out = sum_l PV_l / rowsum_l
    (reciprocal + per-column broadcast multiply + adds on DVE).
  - The per-(duo, query-block) units are software-pipelined one deep
    (QK+exp of unit u before PV of unit u-1) so the Scalar engine — the
    bottleneck at ~1 exp/lane/cycle — never starves while the PE drains the
    PV backlog. Unit order and 512-column-chunked input DMAs minimize the
    startup and drain tails.

TimelineSim cost model: ~130 us/core (Scalar/exp-bound at ~116 us busy;
PE ~72 us modeled, DVE ~40 us, Pool ~13 us).
"""
import numpy as np
import ml_dtypes

B, H, T, DH = 2, 16, 2048, 64
C = H * DH
T1, T2 = 512, 128
NCORES = 8
PAIRS = 4          # (b, h) pairs per core
DUOS = 2           # pairs are packed two-per-SBUF-tile
QB = T // 512      # 512-wide query blocks
NJ = (T // 128, T1 // 128, T2 // 128)

TRACE = False          # set by test.py for profiling runs
LAST_RESULT = None     # BassKernelResults from the most recent run

_NC_CACHE = {}


def _build_nc(w):
    import concourse.bass as bass
    from concourse import bacc
    import concourse.tile as tile
    import concourse.mybir as mybir
    from contextlib import ExitStack

    BF16 = mybir.dt.bfloat16
    F32 = mybir.dt.float32
    Exp = mybir.ActivationFunctionType.Exp

    nc = bacc.Bacc("TRN2", target_bir_lowering=False)
    qt = nc.dram_tensor("qt", [DUOS, 128, T], BF16, kind="ExternalInput")
    k0t = nc.dram_tensor("k0t", [DUOS, 128, T], BF16, kind="ExternalInput")
    k1t = nc.dram_tensor("k1t", [DUOS, 128, T1], BF16, kind="ExternalInput")
    k2t = nc.dram_tensor("k2t", [DUOS, 128, T2], BF16, kind="ExternalInput")
    v0 = nc.dram_tensor("v0", [PAIRS, 128, NJ[0], 65], BF16, kind="ExternalInput")
    v1 = nc.dram_tensor("v1", [PAIRS, 128, NJ[1], 65], BF16, kind="ExternalInput")
    v2 = nc.dram_tensor("v2", [PAIRS, 128, NJ[2], 65], BF16, kind="ExternalInput")
    tri = nc.dram_tensor("tri", [128, 128], BF16, kind="ExternalInput")
    out = nc.dram_tensor("out", [PAIRS, 128, T // 128, DH], F32, kind="ExternalOutput")

    with tile.TileContext(nc) as tc, ExitStack() as ctx:
        const = ctx.enter_context(tc.tile_pool(name="const", bufs=1))
        qkp = ctx.enter_context(tc.tile_pool(name="qkp", bufs=2, space="PSUM"))
        pvp = ctx.enter_context(tc.tile_pool(name="pvp", bufs=4, space="PSUM"))
        pts = ctx.enter_context(tc.tile_pool(name="pts", bufs=44))
        outp = ctx.enter_context(tc.tile_pool(name="outp", bufs=4))
        small = ctx.enter_context(tc.tile_pool(name="small", bufs=8))

        def load(dram_ap, shape, tag):
            t = const.tile(shape, BF16, tag=tag, name=tag)
            nc.sync.dma_start(out=t, in_=dram_ap)
            return t

        # Big Q/K tensors are DMA'd in 512-column chunks so the first
        # query-block's matmuls can start before the full tensors land.
        def load_chunked(dram_ap, shape, tag, chunk=512):
            t = const.tile(shape, BF16, tag=tag, name=tag)
            for c0 in range(0, shape[1], chunk):
                nc.sync.dma_start(out=t[:, c0:c0 + chunk],
                                  in_=dram_ap[:, c0:c0 + chunk])
            return t

        # Emission order = need order: the first unit is (d=0, qb=0), which
        # needs only qt0/k0t0 column chunk 0 plus k1t/k2t of duo 0. V tiles go
        # on the gpsimd SWDGE queue so they stream in parallel with the
        # HWDGE-queued Q/K chunks.
        sb_tri = load(tri[:], [128, 128], "tri")
        def alloc(shape, tag):
            return const.tile(shape, BF16, tag=tag, name=tag)
        sb_qt = [alloc([128, T], f"qt{d}") for d in range(DUOS)]
        sb_kt = [[alloc([128, T], f"k0t{d}") for d in range(DUOS)],
                 [alloc([128, T1], f"k1t{d}") for d in range(DUOS)],
                 [alloc([128, T2], f"k2t{d}") for d in range(DUOS)]]
        sb_v = [[alloc([128, NJ[0], 65], f"v0{p}") for p in range(PAIRS)],
                [alloc([128, NJ[1], 65], f"v1{p}") for p in range(PAIRS)],
                [alloc([128, NJ[2], 65], f"v2{p}") for p in range(PAIRS)]]

        def dma_cols(t, dram_ap, c0, c1, engine=None):
            (engine or nc.sync).dma_start(out=t[:, c0:c1], in_=dram_ap[:, c0:c1])

        for d in range(DUOS):
            # unit (d, 0): first 512 cols of qt/k0t + all of k1t/k2t.
            # k0t's first chunk is split so j=0's 128 columns land first and
            # the very first QK matmul can issue as early as possible.
            dma_cols(sb_kt[0][d], k0t[d], 0, 128)
            dma_cols(sb_qt[d], qt[d], 0, 512)
            dma_cols(sb_kt[0][d], k0t[d], 128, 512)
            nc.sync.dma_start(out=sb_kt[1][d], in_=k1t[d])
            nc.sync.dma_start(out=sb_kt[2][d], in_=k2t[d])
            for p in (2 * d, 2 * d + 1):
                nc.gpsimd.dma_start(out=sb_v[0][p], in_=v0[p])
                nc.gpsimd.dma_start(out=sb_v[1][p], in_=v1[p])
                nc.gpsimd.dma_start(out=sb_v[2][p], in_=v2[p])
            # remaining qt/k0t chunks (needed from unit (d, 3) onwards)
            for c0 in range(512, T, 512):
                dma_cols(sb_kt[0][d], k0t[d], c0, c0 + 512)
            for c0 in range(512, T, 512):
                dma_cols(sb_qt[d], qt[d], c0, c0 + 512)

        def unit_jblocks(qb):
            return ([(0, j) for j in range(4 * qb + 4)]
                    + [(1, j) for j in range(NJ[1])]
                    + [(2, j) for j in range(NJ[2])])

        def phase1(d, qb):
                jblocks = unit_jblocks(qb)
                # ---- Phase 1: S^T = K^T.T @ Q^T per j-block, exp -> P^T
                pt_tiles = {}
                for (l, j) in jblocks:
                    # Diagonal L0 blocks: columns left of the diagonal 128-col
                    # sub-block are fully causal-masked and never read by PV,
                    # so compute/exp only the [f0, 512) column range.
                    f0 = 128 * (j - 4 * qb) if (l == 0 and j >= 4 * qb) else 0
                    sp = qkp.tile([128, 2, 512], F32, tag="qk", name="qk")
                    for half in range(2):
                        nc.tensor.matmul(
                            out=sp[:, half, f0:],
                            lhsT=sb_kt[l][d][64 * half:64 * half + 64,
                                             128 * j:128 * j + 128],
                            rhs=sb_qt[d][64 * half:64 * half + 64,
                                         512 * qb + f0:512 * qb + 512],
                            start=True, stop=True,
                        )
                    pt = pts.tile([128, 2, 512], BF16, tag="pt", name="pt")
                    nc.scalar.activation(out=pt[:, :, f0:], in_=sp[:, :, f0:],
                                         func=Exp, scale=DH ** -0.5)
                    if l == 0 and j >= 4 * qb:
                        for half in range(2):
                            s = pt[:, half, f0:f0 + 128]
                            nc.vector.tensor_mul(out=s, in0=s, in1=sb_tri)
                    pt_tiles[(l, j)] = pt
                return pt_tiles

        def phase2(d, qb, pt_tiles):
                jblocks = unit_jblocks(qb)
                # ---- Phase 2: PV accumulation + combine, per pair half
                for half in range(2):
                    p = 2 * d + half
                    pvt = [pvp.tile([128, 4, 65], F32, tag="pv", name=f"pv{_l}") for _l in range(3)]
                    for (l, j) in jblocks:
                        pt = pt_tiles[(l, j)]
                        # One accumulation group per PSUM bank (= per level):
                        # start zeroes the whole 2KB zero region, so only the
                        # very first matmul into the bank may set start=True.
                        lvl_last_j = (4 * qb + 3) if l == 0 else NJ[l] - 1
                        for c in range(4):
                            if l == 0 and j > 4 * qb + c:
                                continue
                            nc.tensor.matmul(
                                out=pvt[l][:, c, :],
                                lhsT=pt[:, half, 128 * c:128 * c + 128],
                                rhs=sb_v[l][p][:, j, :],
                                start=(j == 0 and c == 0),
                                stop=(j == lvl_last_j and c == 3),
                            )
                    osb = outp.tile([128, 4, DH], F32, tag="osb", name="osb")
                    for l in range(3):
                        rc = small.tile([128, 4, 1], F32, tag="rc", name="rc")
                        nc.vector.reciprocal(out=rc[:, :, 0], in_=pvt[l][:, :, 64])
                        dst = osb if l == 0 else outp.tile([128, 4, DH], F32, tag="tmp", name="tmp")
                        nc.vector.tensor_mul(
                            out=dst, in0=pvt[l][:, :, 0:64],
                            in1=rc.broadcast_to([128, 4, DH]))
                        if l > 0:
                            nc.vector.tensor_add(out=osb, in0=osb, in1=dst)
                    nc.sync.dma_start(out=out[p][:, 4 * qb:4 * qb + 4, :],
                                      in_=osb)

        # Software pipeline: run unit u's QK+exp before unit u-1's PV, so the
        # Scalar engine always has a full unit of exp work queued while the
        # PE drains the previous unit's PV backlog. Unit order puts a small
        # unit first (fast start after partial DMA) and a small-ish one last
        # (short PV tail after the final exp).
        units = [(0, 0), (0, 3), (0, 2), (0, 1), (1, 3), (1, 2), (1, 1), (1, 0)]
        tiles = {}
        pending = []
        for i, u in enumerate(units):
            tiles[u] = phase1(*u)
            if i < len(units) - 2:
                if pending:
                    v = pending.pop(0)
                    phase2(v[0], v[1], tiles.pop(v))
                pending.append(u)
            else:
                # run the last two units' QK+exp back-to-back so the Scalar
                # engine has a full runway while the PE drains the PV backlog
                pending.append(u)
        for v in pending:
            phase2(v[0], v[1], tiles.pop(v))
    nc.compile()
    return nc


def _prepare(inputs):
    bf = ml_dtypes.bfloat16
    Q = np.asarray(inputs["Q"], np.float32)
    Ks = [np.asarray(inputs[k], np.float32) for k in ("K0", "K1", "K2")]
    Vs = [np.asarray(inputs[k], np.float32) for k in ("V0", "V1", "V2")]
    level_w = np.asarray(inputs["level_w"], np.float64)
    e = np.exp(level_w - level_w.max())
    w = (e / e.sum()).astype(np.float64)

    # Host-side layout for sharding: per-head transposed Q/K ([64, Tm]) and
    # s-tiled V with a ones column ([128, nj, 65]).
    QT = np.ascontiguousarray(Q.transpose(0, 1, 3, 2)).astype(bf)  # [B,H,64,T]
    KTs = []
    for Kl in Ks:
        Tm = Kl.shape[1]
        Kh = Kl.reshape(B, Tm, H, DH).transpose(0, 2, 3, 1)  # [B,H,64,Tm]
        KTs.append(np.ascontiguousarray(Kh).astype(bf))
    Vps = []
    for lvl, Vl in enumerate(Vs):
        Tm = Vl.shape[1]
        Vl = Vl * np.float32(w[lvl])   # fold level weight into V (exact in fp32)
        Vh = Vl.reshape(B, Tm, H, DH).transpose(0, 2, 1, 3)  # [B,H,Tm,64]
        vp = np.ones((B, H, Tm // 128, 128, 65), np.float32)
        vp[..., :64] = Vh.reshape(B, H, Tm // 128, 128, DH)
        # -> [B, H, 128(p), nj, 65]
        Vps.append(np.ascontiguousarray(vp.transpose(0, 1, 3, 2, 4)).astype(bf))
    tri = (np.arange(128)[:, None] <= np.arange(128)[None, :]).astype(bf)

    in_maps = []
    for core in range(NCORES):
        m = {
            "qt": np.empty((DUOS, 128, T), bf),
            "k0t": np.empty((DUOS, 128, T), bf),
            "k1t": np.empty((DUOS, 128, T1), bf),
            "k2t": np.empty((DUOS, 128, T2), bf),
            "v0": np.empty((PAIRS, 128, NJ[0], 65), bf),
            "v1": np.empty((PAIRS, 128, NJ[1], 65), bf),
            "v2": np.empty((PAIRS, 128, NJ[2], 65), bf),
            "tri": tri,
        }
        for p in range(PAIRS):
            g = PAIRS * core + p
            b, h = divmod(g, H)
            d, half = divmod(p, 2)
            sl = slice(64 * half, 64 * half + 64)
            m["qt"][d, sl] = QT[b, h]
            m["k0t"][d, sl] = KTs[0][b, h]
            m["k1t"][d, sl] = KTs[1][b, h]
            m["k2t"][d, sl] = KTs[2][b, h]
            m["v0"][p] = Vps[0][b, h]
            m["v1"][p] = Vps[1][b, h]
            m["v2"][p] = Vps[2][b, h]
        in_maps.append(m)

    return in_maps, w


def kernel(**inputs):
    global LAST_RESULT
    from concourse.bass_utils import run_bass_kernel_spmd

    in_maps, w = _prepare(inputs)
    key = tuple(np.asarray(w, np.float64).tolist())
    if key not in _NC_CACHE:
        _NC_CACHE[key] = _build_nc(w)
    nc = _NC_CACHE[key]

    try:
        res = run_bass_kernel_spmd(nc, in_maps, core_ids=list(range(NCORES)),
                                   trace=TRACE)
    except (ImportError, ModuleNotFoundError):
        # axon build without the NTFF profiling hook — run without trace
        res = run_bass_kernel_spmd(nc, in_maps, core_ids=list(range(NCORES)),
                                   trace=False)
    LAST_RESULT = res

    outs = np.stack([np.asarray(r["out"]) for r in res.results])  # [8,4,128,16,64]
    O = outs.transpose(0, 1, 3, 2, 4).reshape(B, H, T, DH)  # t = 128*n + pp
    return np.ascontiguousarray(O.transpose(0, 2, 1, 3)).reshape(B, T, C).astype(np.float32)
